# Initial kernel scaffold
#
"""3-layer GraphSAGE (mean aggregation) on 8 TRN2 NeuronCores.

Strategy (graph/data parallel, per the sharding hint):
  - Nodes are sharded by contiguous range across the 8 cores; each core owns
    the segment-sum targets (edge destinations) in its range.
  - Edge messages are gathered with dma_gather from a replicated bf16 node
    table in HBM (x for layer 0, the AllGather'ed hidden state after). The
    table is split in two halves so int16 gather indices suffice; gathers are
    round-robined over the 4 SWDGE queues so Q7 descriptor generation (the
    dominant cost) overlaps across core pairs.
  - Segment sum runs on the TensorEngine: edges are host-sorted by
    (dst tile, table half), and a host-baked selection matrix S (one-hot dst
    columns scaled by 1/deg) is streamed from HBM;
    agg[dst,:] = sum_chunks S_chunk.T @ msgs_chunk accumulates in PSUM per
    128-node dst tile.
  - h = mean @ Wl + x @ Wr + b is computed per dst tile (PE), with PE
    transposes producing the lhsT operands.
  - After layers 0/1 the local relu'd shard is AllGather'ed (bf16) so every
    core can gather next-layer messages from the full node table.
"""

import sys

sys.path.insert(0, "/opt/trn_rl_repo")

import numpy as np
import ml_dtypes

import concourse.bass as bass
import concourse.bacc as bacc
import concourse.mybir as mybir
import concourse.tile as tile
from concourse.bass_utils import run_bass_kernel_spmd
from concourse.masks import make_identity

BF16 = ml_dtypes.bfloat16
P = 128


class Cfg:
    def __init__(self, n_nodes=50000, dim=256, n_layers=3, n_cores=8,
                 batch_tiles=4, max_gather_chunks=12):
        assert n_nodes % n_cores == 0
        self.N = n_nodes
        self.D = dim
        self.L = n_layers
        self.C = n_cores
        self.NS = n_nodes // n_cores          # nodes per core
        self.T = (self.NS + P - 1) // P       # dst tiles per core
        self.NSP = self.T * P                 # padded nodes per core
        self.HALF = (n_nodes + 1) // 2        # gather table split (int16 idx)
        assert self.HALF < 32768
        self.BATCH = batch_tiles              # dst tiles per PSUM batch
        self.MAXGC = max_gather_chunks        # chunks (of 128 tokens) per dma_gather
        self.KC = dim // P                    # feature k-chunks (2 for D=256)


class Schedule:
    """Compile-time token-stream schedule, shared by all cores."""

    def __init__(self, cfg, counts):
        # counts: [C, T, 2] real-edge counts per (core, dst tile, table half)
        self.cfg = cfg
        mx = counts.max(axis=0)                         # [T, 2]
        self.M = ((mx + P - 1) // P) * P                # padded slots per (t, h)
        empty = self.M.sum(axis=1) == 0                 # tiles with no edges at all
        self.M[empty, 0] = P                            # still need the x@Wr+b path
        self.group_off = np.zeros((cfg.T, 2), np.int64)  # token offset of group (t,h)
        self.segments = []  # (h, tok_off, ntok, [(t, k0_chunks, nk_chunks)], batch_idx)
        tok = 0
        t0 = 0
        bi = 0
        while t0 < cfg.T:
            tiles = range(t0, min(t0 + cfg.BATCH, cfg.T))
            for h in (0, 1):
                seg_tiles = []
                seg_off = tok
                for t in tiles:
                    m = int(self.M[t, h])
                    if m == 0:
                        continue
                    self.group_off[t, h] = tok
                    seg_tiles.append((t, (tok - seg_off) // P, m // P))
                    tok += m
                if seg_tiles:
                    self.segments.append((h, seg_off, tok - seg_off, seg_tiles, bi))
            t0 += cfg.BATCH
            bi += 1
        self.TOT = tok                                   # total tokens per layer
        assert self.TOT % P == 0


def _preprocess(cfg, edge_src, edge_dst):
    """Sort/pad edges per core; build idx streams and baked S matrices."""
    src = np.asarray(edge_src).astype(np.int64)
    dst = np.asarray(edge_dst).astype(np.int64)
    deg = np.bincount(dst, minlength=cfg.N)
    inv = (1.0 / np.maximum(deg, 1)).astype(np.float32)

    core = dst // cfg.NS
    percore = []
    counts = np.zeros((cfg.C, cfg.T, 2), np.int64)
    for c in range(cfg.C):
        m = core == c
        s = src[m]
        dl = dst[m] - c * cfg.NS
        t = dl // P
        h = (s >= cfg.HALF).astype(np.int64)
        order = np.lexsort((h, t))
        s, dl, t, h = s[order], dl[order], t[order], h[order]
        np.add.at(counts[c], (t, h), 1)
        percore.append((s, dl, t, h))

    sched = Schedule(cfg, counts)
    TOT = sched.TOT
    nch = TOT // P

    idx_streams = []
    s_streams = []
    for c in range(cfg.C):
        s, dl, t, h = percore[c]
        # rank within (t, h) group (edges already sorted by (t, h))
        gid = t * 2 + h
        changes = np.empty(len(gid), bool)
        if len(gid):
            changes[0] = True
            changes[1:] = gid[1:] != gid[:-1]
        starts = np.flatnonzero(changes)
        rank = np.arange(len(gid)) - np.repeat(starts, np.diff(np.append(starts, len(gid))))
        tok = sched.group_off[t, h] + rank              # token slot per edge

        idx_val = np.zeros(TOT, np.int16)               # pad -> row 0 (real row, zeroed by S)
        idx_val[tok] = (s % cfg.HALF).astype(np.int16)

        S = np.zeros((nch, P, P), BF16)
        S[tok // P, tok % P, dl % P] = inv[dl + c * cfg.NS]

        # wrapped idx layout per gather segment, then replicated to 128 partitions
        wrapped_cols = []
        for (hh, off, ntok, _tiles, _bi) in sched.segments:
            seg = idx_val[off:off + ntok]
            wrapped_cols.append(seg.reshape(ntok // 16, 16).T)
        wrapped = np.concatenate(wrapped_cols, axis=1)   # [16, TOT//16]
        idx_streams.append(np.ascontiguousarray(np.tile(wrapped, (8, 1))))

        # S as [128, TOT] (chunk ch occupies cols ch*128..(ch+1)*128)
        s_streams.append(np.ascontiguousarray(S.transpose(1, 0, 2).reshape(P, nch * P)))

    return sched, idx_streams, s_streams


def _gather_pieces(sched, cfg):
    """Split each segment into dma_gather pieces of <= MAXGC chunks.

    Returns list per segment: [(piece_chunk_off_in_seg, piece_nchunks)].
    """
    out = []
    for (_h, _off, ntok, _tiles, _bi) in sched.segments:
        nk = ntok // P
        pieces = []
        k = 0
        while k < nk:
            pk = min(cfg.MAXGC, nk - k)
            pieces.append((k, pk))
            k += pk
        out.append(pieces)
    return out


def _build(cfg, sched):
    nc = bacc.Bacc("TRN2", target_bir_lowering=False, debug=False,
                   num_devices=cfg.C, num_swdge_queues=4)
    dt = mybir.dt
    N, D, NS, NSP, T, KC = cfg.N, cfg.D, cfg.NS, cfg.NSP, cfg.T, cfg.KC
    TOT = sched.TOT

    # ---- I/O ----
    x_lo = nc.dram_tensor("x_lo", [cfg.HALF, D], dt.bfloat16, kind="ExternalInput")
    x_hi = nc.dram_tensor("x_hi", [N - cfg.HALF, D], dt.bfloat16, kind="ExternalInput")
    xT_loc = nc.dram_tensor("xT_loc", [P, KC, NSP], dt.bfloat16, kind="ExternalInput")
    idx16 = nc.dram_tensor("idx16", [P, TOT // 16], dt.int16, kind="ExternalInput")
    s_stream = nc.dram_tensor("s_stream", [P, TOT], dt.bfloat16, kind="ExternalInput")
    w_in = {}
    for l in range(cfg.L):
        w_in[("Wl", l)] = nc.dram_tensor(f"Wl{l}b", [KC, P, D], dt.bfloat16, kind="ExternalInput")
        w_in[("Wr", l)] = nc.dram_tensor(f"Wr{l}b", [KC, P, D], dt.bfloat16, kind="ExternalInput")
        w_in[("b", l)] = nc.dram_tensor(f"b{l}b", [1, D], dt.bfloat16, kind="ExternalInput")
    out_ext = nc.dram_tensor("out", [NS, cfg.L, D], dt.float32, kind="ExternalOutput")

    # ---- internal DRAM ----
    ag_in = [nc.dram_tensor(f"ag_in{l}", [NS, D], dt.bfloat16) for l in range(cfg.L - 1)]
    h_full = [nc.dram_tensor(f"h_full{l}", [N, D], dt.bfloat16, addr_space="Shared")
              for l in range(cfg.L - 1)]

    pieces_per_seg = _gather_pieces(sched, cfg)

    with tile.TileContext(nc) as tc:
        with (
            tc.tile_pool(name="const", bufs=1) as constp,
            tc.tile_pool(name="sbuf", bufs=2) as sb,
            tc.tile_pool(name="msgp", bufs=9) as msgp,
            tc.tile_pool(name="psum", bufs=2, space="PSUM") as ps,
            tc.tile_pool(name="aggp", bufs=cfg.BATCH, space="PSUM") as aggps,
        ):
            # persistent constants
            ident = constp.tile([P, P], dt.bfloat16, tag="ident")
            make_identity(nc, ident[:, :])
            ones_row = constp.tile([1, P], dt.bfloat16, tag="ones")
            nc.gpsimd.memset(ones_row[:, :], 1.0)
            idx_sb = constp.tile([P, TOT // 16], dt.int16, tag="idx")
            nc.sync.dma_start(out=idx_sb[:, :], in_=idx16[:, :])
            w_sb = {}
            for l in range(cfg.L):
                for nm in ("Wl", "Wr"):
                    w = constp.tile([P, KC, D], dt.bfloat16, tag=f"{nm}{l}")
                    for k in range(KC):
                        nc.sync.dma_start(out=w[:, k, :], in_=w_in[(nm, l)][k, :, :])
                    w_sb[(nm, l)] = w
                bt = constp.tile([1, D], dt.bfloat16, tag=f"b{l}")
                nc.sync.dma_start(out=bt[:, :], in_=w_in[("b", l)][:, :])
                w_sb[("b", l)] = bt
            # hprev transposed, ping-pong
            hT = [constp.tile([P, KC, NSP], dt.bfloat16, tag=f"hT{i}",
                              name=f"hT{i}") for i in range(2)]
            for k in range(KC):
                nc.sync.dma_start(out=hT[0][:, k, :], in_=xT_loc[:, k, :])

            gq = [0]
            for l in range(cfg.L):
                if l == 0:
                    tables = (x_lo[:, :], x_hi[:, :])
                else:
                    hf = h_full[l - 1]
                    tables = (hf[0:cfg.HALF, :], hf[cfg.HALF:N, :])
                hT_cur = hT[l % 2]
                hT_nxt = hT[(l + 1) % 2]

                agg_of = {}
                first_mm = {}
                nseg = len(sched.segments)
                for si, (h, tok_off, ntok, seg_tiles, bi) in enumerate(sched.segments):
                    # batch boundary: allocate PSUM accumulators at first segment of batch
                    for (t, k0, nk) in seg_tiles:
                        if t not in agg_of:
                            agg_of[t] = aggps.tile([P, D], dt.float32, tag="agg",
                                                   name=f"agg_l{l}_t{t}")
                            first_mm[t] = True
                    # gather pieces (round-robin the 4 SWDGE queues so
                    # descriptor generation parallelizes across Q7 core pairs)
                    msg_tiles = []
                    for (pk0, pnk) in pieces_per_seg[si]:
                        mt = msgp.tile([P, cfg.MAXGC, D], dt.bfloat16, tag="msg")
                        ntk = pnk * P
                        c0 = (tok_off + pk0 * P) // 16
                        nc.gpsimd.dma_gather(
                            mt[:, 0:pnk, :],
                            tables[h],
                            idx_sb[:, c0:c0 + ntk // 16],
                            ntk, ntk, D,
                            single_packet=False,
                            queue_num=gq[0] % 4,
                        )
                        gq[0] += 1
                        msg_tiles.append(mt)
                    # S columns for this segment
                    s_sb = sb.tile([P, ntok], dt.bfloat16, tag="sseg")
                    nc.sync.dma_start(
                        out=s_sb[:, :], in_=s_stream[:, tok_off:tok_off + ntok])
                    # aggregation matmuls
                    last_of_tile = {}
                    for (t, k0, nk) in seg_tiles:
                        last_of_tile[t] = (h == 1) or sched.M[t, 1] == 0
                    for (t, k0, nk) in seg_tiles:
                        for j in range(nk):
                            ch = k0 + j
                            pi = ch // cfg.MAXGC
                            loc = ch - pieces_per_seg[si][pi][0]
                            nc.tensor.matmul(
                                agg_of[t][:, :],
                                lhsT=s_sb[:, ch * P:(ch + 1) * P],
                                rhs=msg_tiles[pi][:, loc, :],
                                start=first_mm[t],
                                stop=last_of_tile[t] and j == nk - 1,
                            )
                            first_mm[t] = False
                    # after the last segment of the batch: finish its tiles
                    batch_done = (si + 1 == nseg) or sched.segments[si + 1][4] != bi
                    if not batch_done:
                        continue
                    for t in sorted(agg_of):
                        aggt = agg_of[t]
                        rows = min(P, NS - t * P)
                        # mean (inv-degree already baked into S) -> bf16
                        mean_sb = sb.tile([P, D], dt.bfloat16, tag="mean")
                        nc.vector.tensor_copy(out=mean_sb[:, :], in_=aggt[:, :])
                        # transpose mean -> meanT (lhsT layout)
                        meanT = sb.tile([P, KC, P], dt.bfloat16, tag="meanT")
                        for k in range(KC):
                            tp = ps.tile([P, P], dt.bfloat16, tag="tp")
                            nc.tensor.transpose(
                                out=tp[:, :], in_=mean_sb[:, k * P:(k + 1) * P],
                                identity=ident[:, :])
                            nc.vector.tensor_copy(out=meanT[:, k, :], in_=tp[:, :])
                        # h = meanT.T @ Wl + hprevT.T @ Wr + b
                        hp = ps.tile([P, D], dt.float32, tag="hp")
                        nc.tensor.matmul(hp[:, :], lhsT=ones_row[:, :],
                                         rhs=w_sb[("b", l)][:, :],
                                         start=True, stop=False)
                        for k in range(KC):
                            nc.tensor.matmul(hp[:, :], lhsT=meanT[:, k, :],
                                             rhs=w_sb[("Wl", l)][:, k, :],
                                             start=False, stop=False)
                        for k in range(KC):
                            nc.tensor.matmul(hp[:, :],
                                             lhsT=hT_cur[:, k, t * P:(t + 1) * P],
                                             rhs=w_sb[("Wr", l)][:, k, :],
                                             start=False, stop=k == KC - 1)
                        # f32 output (pre-relu)
                        hout = sb.tile([P, D], dt.float32, tag="hout")
                        nc.vector.tensor_copy(out=hout[:, :], in_=hp[:, :])
                        nc.sync.dma_start(
                            out=out_ext[t * P:t * P + rows, l, :],
                            in_=hout[0:rows, :])
                        if l < cfg.L - 1:
                            hrelu = sb.tile([P, D], dt.bfloat16, tag="hrelu")
                            nc.scalar.activation(
                                out=hrelu[:, :], in_=hp[:, :],
                                func=mybir.ActivationFunctionType.Relu)
                            nc.sync.dma_start(
                                out=ag_in[l][t * P:t * P + rows, :],
                                in_=hrelu[0:rows, :])
                            for k in range(KC):
                                tq = ps.tile([P, P], dt.bfloat16, tag="tp")
                                nc.tensor.transpose(
                                    out=tq[:, :],
                                    in_=hrelu[:, k * P:(k + 1) * P],
                                    identity=ident[:, :])
                                nc.vector.tensor_copy(
                                    out=hT_nxt[:, k, t * P:(t + 1) * P],
                                    in_=tq[:, :])
                    agg_of = {}
                    first_mm = {}

                if l < cfg.L - 1:
                    nc.gpsimd.collective_compute(
                        "AllGather",
                        mybir.AluOpType.bypass,
                        replica_groups=[list(range(cfg.C))],
                        ins=[ag_in[l][:, :]],
                        outs=[h_full[l][:, :]],
                    )

    nc.compile()
    return nc


def _prepare_inputs(cfg, inputs):
    """Host-side shard/pack. Returns (sched, per-core input maps)."""
    x = np.asarray(inputs["x"], np.float32)
    sched, idx_streams, s_streams = _preprocess(
        cfg, inputs["edge_src"], inputs["edge_dst"])

    x_bf = x.astype(BF16)
    x_lo = np.ascontiguousarray(x_bf[:cfg.HALF])
    x_hi = np.ascontiguousarray(x_bf[cfg.HALF:])

    in_maps = []
    for c in range(cfg.C):
        xc = x_bf[c * cfg.NS:(c + 1) * cfg.NS]           # [NS, D]
        xT = np.zeros((cfg.D, cfg.NSP), BF16)
        xT[:, :cfg.NS] = xc.T
        xT = np.ascontiguousarray(
            xT.reshape(cfg.KC, P, cfg.NSP).transpose(1, 0, 2))
        m = {
            "x_lo": x_lo,
            "x_hi": x_hi,
            "xT_loc": xT,
            "idx16": idx_streams[c],
            "s_stream": s_streams[c],
        }
        for l in range(cfg.L):
            wl = np.asarray(inputs[f"Wl{l}"], np.float32).astype(BF16)
            wr = np.asarray(inputs[f"Wr{l}"], np.float32).astype(BF16)
            bb = np.asarray(inputs[f"b{l}"], np.float32).astype(BF16)
            m[f"Wl{l}b"] = np.ascontiguousarray(wl.reshape(cfg.KC, P, cfg.D))
            m[f"Wr{l}b"] = np.ascontiguousarray(wr.reshape(cfg.KC, P, cfg.D))
            m[f"b{l}b"] = np.ascontiguousarray(bb.reshape(1, cfg.D))
        in_maps.append(m)
    return sched, in_maps


_CACHE = {}


def run(inputs, cfg=None, trace=False):
    cfg = cfg or Cfg()
    sched, in_maps = _prepare_inputs(cfg, inputs)
    key = (cfg.N, cfg.D, cfg.C, tuple(sched.M.ravel()))
    if key not in _CACHE:
        _CACHE[key] = _build(cfg, sched)
    nc = _CACHE[key]
    res = run_bass_kernel_spmd(nc, in_maps, list(range(cfg.C)), trace=trace)
    out = np.concatenate([res.results[c]["out"] for c in range(cfg.C)], axis=0)
    return out, res


def kernel(**inputs):
    out, _ = run(inputs)
    return out



# revision 1
# speedup vs baseline: 1.1854x; 1.1854x over previous
"""3-layer GraphSAGE (mean aggregation) on 8 TRN2 NeuronCores.

Strategy (graph/data parallel, per the sharding hint):
  - Nodes are sharded by contiguous range across the 8 cores; each core owns
    the segment-sum targets (edge destinations) in its range.
  - Edge messages are gathered with dma_gather from a replicated bf16 node
    table in HBM (x for layer 0, the AllGather'ed hidden state after). The
    table is split in two halves so int16 gather indices suffice; gathers are
    round-robined over the 4 SWDGE queues so Q7 descriptor generation (the
    dominant cost) overlaps across core pairs.
  - Segment sum runs on the TensorEngine: edges are host-sorted by
    (dst tile, table half), and a host-baked selection matrix S (one-hot dst
    columns scaled by 1/deg) is streamed from HBM;
    agg[dst,:] = sum_chunks S_chunk.T @ msgs_chunk accumulates in PSUM per
    128-node dst tile.
  - h = mean @ Wl + x @ Wr + b is computed per dst tile (PE), with PE
    transposes producing the lhsT operands.
  - After layers 0/1 the local relu'd shard is AllGather'ed (bf16) so every
    core can gather next-layer messages from the full node table.
"""

import sys

sys.path.insert(0, "/opt/trn_rl_repo")

import numpy as np
import ml_dtypes

import concourse.bass as bass
import concourse.bacc as bacc
import concourse.mybir as mybir
import concourse.tile as tile
from concourse.bass_utils import run_bass_kernel_spmd
from concourse.masks import make_identity

BF16 = ml_dtypes.bfloat16
P = 128


class Cfg:
    def __init__(self, n_nodes=50000, dim=256, n_layers=3, n_cores=8,
                 batch_tiles=4, max_gather_chunks=12):
        assert n_nodes % n_cores == 0
        self.N = n_nodes
        self.D = dim
        self.L = n_layers
        self.C = n_cores
        self.NS = n_nodes // n_cores          # nodes per core
        self.T = (self.NS + P - 1) // P       # dst tiles per core
        self.NSP = self.T * P                 # padded nodes per core
        self.HALF = (n_nodes + 1) // 2        # gather table split (int16 idx)
        assert self.HALF < 32768
        self.BATCH = batch_tiles              # dst tiles per PSUM batch
        self.MAXGC = max_gather_chunks        # chunks (of 128 tokens) per dma_gather
        self.KC = dim // P                    # feature k-chunks (2 for D=256)


class Schedule:
    """Compile-time token-stream schedule, shared by all cores."""

    def __init__(self, cfg, counts):
        # counts: [C, T, 2] real-edge counts per (core, dst tile, table half)
        self.cfg = cfg
        mx = counts.max(axis=0)                         # [T, 2]
        self.M = ((mx + P - 1) // P) * P                # padded slots per (t, h)
        empty = self.M.sum(axis=1) == 0                 # tiles with no edges at all
        self.M[empty, 0] = P                            # still need the x@Wr+b path
        self.group_off = np.zeros((cfg.T, 2), np.int64)  # token offset of group (t,h)
        self.segments = []  # (h, tok_off, ntok, [(t, k0_chunks, nk_chunks)], batch_idx)
        tok = 0
        t0 = 0
        bi = 0
        while t0 < cfg.T:
            tiles = range(t0, min(t0 + cfg.BATCH, cfg.T))
            for h in (0, 1):
                seg_tiles = []
                seg_off = tok
                for t in tiles:
                    m = int(self.M[t, h])
                    if m == 0:
                        continue
                    self.group_off[t, h] = tok
                    seg_tiles.append((t, (tok - seg_off) // P, m // P))
                    tok += m
                if seg_tiles:
                    self.segments.append((h, seg_off, tok - seg_off, seg_tiles, bi))
            t0 += cfg.BATCH
            bi += 1
        self.TOT = tok                                   # total tokens per layer
        assert self.TOT % P == 0


def _preprocess(cfg, edge_src, edge_dst):
    """Sort/pad edges per core; build idx streams and baked S matrices."""
    src = np.asarray(edge_src).astype(np.int64)
    dst = np.asarray(edge_dst).astype(np.int64)
    deg = np.bincount(dst, minlength=cfg.N)
    inv = (1.0 / np.maximum(deg, 1)).astype(np.float32)

    core = dst // cfg.NS
    percore = []
    counts = np.zeros((cfg.C, cfg.T, 2), np.int64)
    for c in range(cfg.C):
        m = core == c
        s = src[m]
        dl = dst[m] - c * cfg.NS
        t = dl // P
        h = (s >= cfg.HALF).astype(np.int64)
        order = np.lexsort((h, t))
        s, dl, t, h = s[order], dl[order], t[order], h[order]
        np.add.at(counts[c], (t, h), 1)
        percore.append((s, dl, t, h))

    sched = Schedule(cfg, counts)
    TOT = sched.TOT
    nch = TOT // P

    idx_streams = []
    s_streams = []
    for c in range(cfg.C):
        s, dl, t, h = percore[c]
        # rank within (t, h) group (edges already sorted by (t, h))
        gid = t * 2 + h
        changes = np.empty(len(gid), bool)
        if len(gid):
            changes[0] = True
            changes[1:] = gid[1:] != gid[:-1]
        starts = np.flatnonzero(changes)
        rank = np.arange(len(gid)) - np.repeat(starts, np.diff(np.append(starts, len(gid))))
        tok = sched.group_off[t, h] + rank              # token slot per edge

        idx_val = np.zeros(TOT, np.int16)               # pad -> row 0 (real row, zeroed by S)
        idx_val[tok] = (s % cfg.HALF).astype(np.int16)

        S = np.zeros((nch, P, P), BF16)
        S[tok // P, tok % P, dl % P] = inv[dl + c * cfg.NS]

        # wrapped idx layout per gather segment, then replicated to 128 partitions
        wrapped_cols = []
        for (hh, off, ntok, _tiles, _bi) in sched.segments:
            seg = idx_val[off:off + ntok]
            wrapped_cols.append(seg.reshape(ntok // 16, 16).T)
        wrapped = np.concatenate(wrapped_cols, axis=1)   # [16, TOT//16]
        idx_streams.append(np.ascontiguousarray(np.tile(wrapped, (8, 1))))

        # S as [128, TOT] (chunk ch occupies cols ch*128..(ch+1)*128)
        s_streams.append(np.ascontiguousarray(S.transpose(1, 0, 2).reshape(P, nch * P)))

    return sched, idx_streams, s_streams


def _gather_pieces(sched, cfg):
    """Split each segment into dma_gather pieces of <= MAXGC chunks.

    Returns list per segment: [(piece_chunk_off_in_seg, piece_nchunks)].
    """
    out = []
    for (_h, _off, ntok, _tiles, _bi) in sched.segments:
        nk = ntok // P
        pieces = []
        k = 0
        while k < nk:
            pk = min(cfg.MAXGC, nk - k)
            pieces.append((k, pk))
            k += pk
        out.append(pieces)
    return out


def _build(cfg, sched):
    nc = bacc.Bacc("TRN2", target_bir_lowering=False, debug=False,
                   num_devices=cfg.C, num_swdge_queues=4)
    dt = mybir.dt
    N, D, NS, NSP, T, KC = cfg.N, cfg.D, cfg.NS, cfg.NSP, cfg.T, cfg.KC
    TOT = sched.TOT

    # ---- I/O ----
    x_lo = nc.dram_tensor("x_lo", [cfg.HALF, D], dt.bfloat16, kind="ExternalInput")
    x_hi = nc.dram_tensor("x_hi", [N - cfg.HALF, D], dt.bfloat16, kind="ExternalInput")
    xT_loc = nc.dram_tensor("xT_loc", [P, KC, NSP], dt.bfloat16, kind="ExternalInput")
    idx16 = nc.dram_tensor("idx16", [P, TOT // 16], dt.int16, kind="ExternalInput")
    s_stream = nc.dram_tensor("s_stream", [P, TOT], dt.bfloat16, kind="ExternalInput")
    w_in = {}
    for l in range(cfg.L):
        w_in[("Wl", l)] = nc.dram_tensor(f"Wl{l}b", [KC, P, D], dt.bfloat16, kind="ExternalInput")
        w_in[("Wr", l)] = nc.dram_tensor(f"Wr{l}b", [KC, P, D], dt.bfloat16, kind="ExternalInput")
        w_in[("b", l)] = nc.dram_tensor(f"b{l}b", [1, D], dt.bfloat16, kind="ExternalInput")
    out_ext = nc.dram_tensor("out", [NS, cfg.L, D], dt.float32, kind="ExternalOutput")

    # ---- internal DRAM ----
    ag_in = [nc.dram_tensor(f"ag_in{l}", [NS, D], dt.bfloat16) for l in range(cfg.L - 1)]
    h_full = [nc.dram_tensor(f"h_full{l}", [N, D], dt.bfloat16, addr_space="Shared")
              for l in range(cfg.L - 1)]

    pieces_per_seg = _gather_pieces(sched, cfg)

    with tile.TileContext(nc) as tc:
        with (
            tc.tile_pool(name="const", bufs=1) as constp,
            tc.tile_pool(name="sbuf", bufs=2) as sb,
            tc.tile_pool(name="msgp", bufs=9) as msgp,
            tc.tile_pool(name="psum", bufs=2, space="PSUM") as ps,
            tc.tile_pool(name="aggp", bufs=cfg.BATCH, space="PSUM") as aggps,
        ):
            # persistent constants
            ident = constp.tile([P, P], dt.bfloat16, tag="ident")
            make_identity(nc, ident[:, :])
            ones_row = constp.tile([1, P], dt.bfloat16, tag="ones")
            nc.gpsimd.memset(ones_row[:, :], 1.0)
            idx_sb = constp.tile([P, TOT // 16], dt.int16, tag="idx")
            nc.sync.dma_start(out=idx_sb[:, :], in_=idx16[:, :])
            w_sb = {}
            for l in range(cfg.L):
                for nm in ("Wl", "Wr"):
                    w = constp.tile([P, KC, D], dt.bfloat16, tag=f"{nm}{l}")
                    for k in range(KC):
                        nc.sync.dma_start(out=w[:, k, :], in_=w_in[(nm, l)][k, :, :])
                    w_sb[(nm, l)] = w
                bt = constp.tile([1, D], dt.bfloat16, tag=f"b{l}")
                nc.sync.dma_start(out=bt[:, :], in_=w_in[("b", l)][:, :])
                w_sb[("b", l)] = bt
            # hprev transposed, ping-pong
            hT = [constp.tile([P, KC, NSP], dt.bfloat16, tag=f"hT{i}",
                              name=f"hT{i}") for i in range(2)]
            for k in range(KC):
                nc.sync.dma_start(out=hT[0][:, k, :], in_=xT_loc[:, k, :])

            gq = [0]
            for l in range(cfg.L):
                if l == 0:
                    tables = (x_lo[:, :], x_hi[:, :])
                else:
                    hf = h_full[l - 1]
                    tables = (hf[0:cfg.HALF, :], hf[cfg.HALF:N, :])
                hT_cur = hT[l % 2]
                hT_nxt = hT[(l + 1) % 2]

                agg_of = {}
                first_mm = {}
                nseg = len(sched.segments)
                for si, (h, tok_off, ntok, seg_tiles, bi) in enumerate(sched.segments):
                    # batch boundary: allocate PSUM accumulators at first segment of batch
                    for (t, k0, nk) in seg_tiles:
                        if t not in agg_of:
                            agg_of[t] = aggps.tile([P, D], dt.float32, tag="agg",
                                                   name=f"agg_l{l}_t{t}")
                            first_mm[t] = True
                    # gather pieces (round-robin the 4 SWDGE queues so
                    # descriptor generation parallelizes across Q7 core pairs)
                    msg_tiles = []
                    for (pk0, pnk) in pieces_per_seg[si]:
                        mt = msgp.tile([P, cfg.MAXGC, D], dt.bfloat16, tag="msg")
                        ntk = pnk * P
                        c0 = (tok_off + pk0 * P) // 16
                        nc.gpsimd.dma_gather(
                            mt[:, 0:pnk, :],
                            tables[h],
                            idx_sb[:, c0:c0 + ntk // 16],
                            ntk, ntk, D,
                            single_packet=False,
                            queue_num=gq[0] % 4,
                        )
                        gq[0] += 1
                        msg_tiles.append(mt)
                    # S columns for this segment
                    s_sb = sb.tile([P, ntok], dt.bfloat16, tag="sseg")
                    nc.sync.dma_start(
                        out=s_sb[:, :], in_=s_stream[:, tok_off:tok_off + ntok])
                    # aggregation matmuls
                    last_of_tile = {}
                    for (t, k0, nk) in seg_tiles:
                        last_of_tile[t] = (h == 1) or sched.M[t, 1] == 0
                    for (t, k0, nk) in seg_tiles:
                        for j in range(nk):
                            ch = k0 + j
                            pi = ch // cfg.MAXGC
                            loc = ch - pieces_per_seg[si][pi][0]
                            nc.tensor.matmul(
                                agg_of[t][:, :],
                                lhsT=s_sb[:, ch * P:(ch + 1) * P],
                                rhs=msg_tiles[pi][:, loc, :],
                                start=first_mm[t],
                                stop=last_of_tile[t] and j == nk - 1,
                            )
                            first_mm[t] = False
                    # after the last segment of the batch: finish its tiles
                    batch_done = (si + 1 == nseg) or sched.segments[si + 1][4] != bi
                    if not batch_done:
                        continue
                    for t in sorted(agg_of):
                        aggt = agg_of[t]
                        rows = min(P, NS - t * P)
                        # mean (inv-degree already baked into S) -> bf16
                        mean_sb = sb.tile([P, D], dt.bfloat16, tag="mean")
                        nc.vector.tensor_copy(out=mean_sb[:, :], in_=aggt[:, :])
                        # transpose mean -> meanT (lhsT layout)
                        meanT = sb.tile([P, KC, P], dt.bfloat16, tag="meanT")
                        for k in range(KC):
                            tp = ps.tile([P, P], dt.bfloat16, tag="tp")
                            nc.tensor.transpose(
                                out=tp[:, :], in_=mean_sb[:, k * P:(k + 1) * P],
                                identity=ident[:, :])
                            nc.vector.tensor_copy(out=meanT[:, k, :], in_=tp[:, :])
                        # h = meanT.T @ Wl + hprevT.T @ Wr + b
                        hp = ps.tile([P, D], dt.float32, tag="hp")
                        nc.tensor.matmul(hp[:, :], lhsT=ones_row[:, :],
                                         rhs=w_sb[("b", l)][:, :],
                                         start=True, stop=False)
                        for k in range(KC):
                            nc.tensor.matmul(hp[:, :], lhsT=meanT[:, k, :],
                                             rhs=w_sb[("Wl", l)][:, k, :],
                                             start=False, stop=False)
                        for k in range(KC):
                            nc.tensor.matmul(hp[:, :],
                                             lhsT=hT_cur[:, k, t * P:(t + 1) * P],
                                             rhs=w_sb[("Wr", l)][:, k, :],
                                             start=False, stop=k == KC - 1)
                        # f32 output (pre-relu)
                        hout = sb.tile([P, D], dt.float32, tag="hout")
                        nc.vector.tensor_copy(out=hout[:, :], in_=hp[:, :])
                        nc.sync.dma_start(
                            out=out_ext[t * P:t * P + rows, l, :],
                            in_=hout[0:rows, :])
                        if l < cfg.L - 1:
                            hrelu = sb.tile([P, D], dt.bfloat16, tag="hrelu")
                            nc.scalar.activation(
                                out=hrelu[:, :], in_=hp[:, :],
                                func=mybir.ActivationFunctionType.Relu)
                            nc.sync.dma_start(
                                out=ag_in[l][t * P:t * P + rows, :],
                                in_=hrelu[0:rows, :])
                            for k in range(KC):
                                tq = ps.tile([P, P], dt.bfloat16, tag="tp")
                                nc.tensor.transpose(
                                    out=tq[:, :],
                                    in_=hrelu[:, k * P:(k + 1) * P],
                                    identity=ident[:, :])
                                nc.vector.tensor_copy(
                                    out=hT_nxt[:, k, t * P:(t + 1) * P],
                                    in_=tq[:, :])
                    agg_of = {}
                    first_mm = {}

                if l < cfg.L - 1:
                    nc.gpsimd.collective_compute(
                        "AllGather",
                        mybir.AluOpType.bypass,
                        replica_groups=[list(range(cfg.C))],
                        ins=[ag_in[l][:, :]],
                        outs=[h_full[l][:, :]],
                    )

    nc.compile()
    return nc


def _prepare_inputs(cfg, inputs):
    """Host-side shard/pack. Returns (sched, per-core input maps)."""
    x = np.asarray(inputs["x"], np.float32)
    sched, idx_streams, s_streams = _preprocess(
        cfg, inputs["edge_src"], inputs["edge_dst"])

    x_bf = x.astype(BF16)
    x_lo = np.ascontiguousarray(x_bf[:cfg.HALF])
    x_hi = np.ascontiguousarray(x_bf[cfg.HALF:])

    in_maps = []
    for c in range(cfg.C):
        xc = x_bf[c * cfg.NS:(c + 1) * cfg.NS]           # [NS, D]
        xT = np.zeros((cfg.D, cfg.NSP), BF16)
        xT[:, :cfg.NS] = xc.T
        xT = np.ascontiguousarray(
            xT.reshape(cfg.KC, P, cfg.NSP).transpose(1, 0, 2))
        m = {
            "x_lo": x_lo,
            "x_hi": x_hi,
            "xT_loc": xT,
            "idx16": idx_streams[c],
            "s_stream": s_streams[c],
        }
        for l in range(cfg.L):
            wl = np.asarray(inputs[f"Wl{l}"], np.float32).astype(BF16)
            wr = np.asarray(inputs[f"Wr{l}"], np.float32).astype(BF16)
            bb = np.asarray(inputs[f"b{l}"], np.float32).astype(BF16)
            m[f"Wl{l}b"] = np.ascontiguousarray(wl.reshape(cfg.KC, P, cfg.D))
            m[f"Wr{l}b"] = np.ascontiguousarray(wr.reshape(cfg.KC, P, cfg.D))
            m[f"b{l}b"] = np.ascontiguousarray(bb.reshape(1, cfg.D))
        in_maps.append(m)
    return sched, in_maps


_CACHE = {}


def run(inputs, cfg=None, trace=False):
    cfg = cfg or Cfg()
    sched, in_maps = _prepare_inputs(cfg, inputs)
    key = (cfg.N, cfg.D, cfg.C, tuple(sched.M.ravel()))
    if key not in _CACHE:
        _CACHE[key] = _build(cfg, sched)
    nc = _CACHE[key]
    res = run_bass_kernel_spmd(nc, in_maps, list(range(cfg.C)), trace=trace)
    out = np.concatenate([res.results[c]["out"] for c in range(cfg.C)], axis=0)
    return out, res


def kernel(**inputs):
    out, _ = run(inputs)
    return out

